# revision 21
# baseline (speedup 1.0000x reference)
"""Causal self-attention (B=4, S=2048, D=1024, single head, fp32) on 8 trn2
NeuronCores.

Sharding: core 2*b + c handles batch b with the parity-c half of the keys
(global key rows 2*i + c), over ALL queries — a flash-attention split over
the key dimension. Each core returns unnormalized softmax numerators
o = sum_k exp(s*scale) v plus per-row l = sum exp; the host combines the
two key-halves exactly (no max subtraction needed: |logits| <~ 6).

SPMD trick: one program serves both parities. The host pair-swaps the rows
of x for odd cores (rows [1,0,3,2,...]), so each core's keys sit at even
row positions and the on-chip stride-2 access pattern is parity-free. The
causal boundary masks (which depend on the parity) ship as a small
per-core input; the host pair-swaps the outputs of odd cores back.

x ships pre-transposed from the host (layout prep is host-side, like the
pair-swap), so the kernel spends no PE/DVE cycles transposing; everything
runs in float32r (full PE rate at free-dim >= 256) directly off DMA —
f32r is bit-identical to f32, so no cast pass is needed. Softmax exp runs
on the scalar engine straight out of PSUM with the causal mask pre-added
into PSUM; attn @ v runs in bf16.
"""
import math
import numpy as np

import concourse.bacc as bacc
import concourse.mybir as mybir
from concourse import tile
from concourse.masks import make_identity
from concourse.bass_utils import run_bass_kernel_spmd

B, S, D = 4, 2048, 1024
P = 128
DT = D // P          # 8 d-tiles (contraction)
ET = D // P          # 8 e-tiles (output feature)
ST = S // P          # 16 s-tiles (sequence)
HKT = ST // 2        # 8 compacted key tiles per core
NQB = S // P         # 16 query blocks
INV_SQRT_D = 1.0 / math.sqrt(D)
NEG = -1e30

F32 = mybir.dt.float32
F32R = mybir.dt.float32r
BF16 = mybir.dt.bfloat16

_CACHED_NC = None


def _chunks(ncols):
    """Split ncols (multiple of 128) into <=512 chunks, all >=256 when
    possible (f32r matmul runs 4 cyc/row below 256 moving rows)."""
    if ncols <= 512:
        return [ncols]
    if ncols <= 896:
        return [ncols - 384, 384]
    return [512, ncols - 512]


def build_nc():
    nc = bacc.Bacc("TRN2", target_bir_lowering=False)
    # x^T shipped chunk-major: block (dt*2+h) holds x^T[dt*128:(dt+1)*128,
    # h*1024:(h+1)*1024] as a contiguous 512KB run for full-rate DMA.
    xt_p = nc.declare_dram_parameter("xt", [DT * 2 * P, S // 2], F32R,
                                     isOutput=False)
    wq_p = nc.declare_dram_parameter("wq", [D, D], F32R, isOutput=False)
    wk_p = nc.declare_dram_parameter("wk", [D, D], F32R, isOutput=False)
    wv_p = nc.declare_dram_parameter("wv", [D, D], F32R, isOutput=False)
    mask_p = nc.declare_dram_parameter("mask", [P, 2, P], F32, isOutput=False)
    o_p = nc.declare_dram_parameter("o", [S, D], F32, isOutput=True)
    l_p = nc.declare_dram_parameter("l", [P, NQB], F32, isOutput=True)
    dbg_p = nc.declare_dram_parameter("dbg", [P, 1], F32, isOutput=True)

    with tile.TileContext(nc) as tc:
        # ---- persistent pools (bottom of SBUF stack) ----
        with (
            tc.tile_pool(name="qT_pool", bufs=1) as qT_pool,
            tc.tile_pool(name="kT_pool", bufs=1) as kT_pool,
            tc.tile_pool(name="v_pool", bufs=1) as v_pool,
            tc.tile_pool(name="const_pool", bufs=1) as const_pool,
        ):
            qT = qT_pool.tile([P, ET, S], F32R)        # [e_p, et, s_q] 64KB/p
            kT = kT_pool.tile([P, ET, HKT * P], F32R)  # [e_p, et, s_k] 32KB/p
            vv = v_pool.tile([P, HKT, D], BF16)        # [s_k_p, st, e] 16KB/p
            ident_bf = const_pool.tile([P, P], BF16)
            mask_sb = const_pool.tile([P, 2, P], F32)
            l_parts = const_pool.tile([P, NQB, 2], F32)
            l_sum = const_pool.tile([P, NQB], F32)
            warm = const_pool.tile([P, 512], BF16)
            dbg_sb = const_pool.tile([P, 1], F32)
            make_identity(nc, ident_bf[:])
            nc.gpsimd.memset(l_parts[:], 0.0)
            nc.gpsimd.memset(warm[:], 0.25)
            nc.sync.dma_start(out=mask_sb[:], in_=mask_p[:])

            # ================= Phase A: x^T + projections =================
            with (
                tc.tile_pool(name="xT_pool", bufs=1) as xT_pool,
                tc.tile_pool(name="stage_pool", bufs=2) as stage_pool,
                tc.tile_pool(name="psA_all", bufs=1, space="PSUM") as psAll,
            ):
                xT = xT_pool.tile([P, DT, S], F32R)    # [d_p, dt, s] 64KB/p
                psb = [psAll.tile([P, 512], F32, tag=f"b{i}", name=f"psb{i}")
                       for i in range(8)]

                # A1: load pre-transposed x. First half-columns go on the
                # sync queue (interleaved with the first wk slices) so A2's
                # ch=0 pass can start as soon as possible; the second half
                # loads in the background on the gpsimd DMA queue.
                wk_t = {}
                for ch in range(2):
                    for et in range(ET):
                        wk_t[(ch, et)] = stage_pool.tile(
                            [P, DT, P], F32R, tag="wr",
                            name=f"wkr{ch}_{et}", bufs=3)
                for et in range(3):
                    nc.sync.dma_start(
                        out=wk_t[(0, et)][:],
                        in_=wk_p[:, et * P:(et + 1) * P].rearrange(
                            "(dt p) e -> p dt e", p=P))
                for dt in range(DT):
                    nc.sync.dma_start(
                        out=xT[:, dt, 0:1024],
                        in_=xt_p[dt * 2 * P:(dt * 2 + 1) * P, :])
                for dt in range(DT):
                    nc.gpsimd.dma_start(
                        out=xT[:, dt, 1024:2048],
                        in_=xt_p[(dt * 2 + 1) * P:(dt * 2 + 2) * P, :])

                # PE warmup: keeps the tensor engine busy (and its clock
                # ramping) while the first x/wk chunks stream in. The psum
                # result feeds a dummy output so it isn't dead-code.
                for w in range(16):
                    nc.tensor.matmul(psb[7][:], ident_bf[:], warm[:],
                                     start=(w == 0), stop=(w == 15))
                nc.vector.tensor_copy(dbg_sb[:], psb[7][:, :1])
                nc.sync.dma_start(out=dbg_p[:], in_=dbg_sb[:])

                # even-position (this core's keys) stride-2 view of xT
                xT_keys = xT.rearrange("p d (s two) -> p d two s", two=2)

                # A2: kT[e, i] = sum_d Wk[d, e] * x_key[i, d].  ch-outer so
                # the first pass only needs the first half of x.
                for ch in range(2):
                    for et in range(ET):
                        wk_r = wk_t[(ch, et)]
                        if not (ch == 0 and et < 3):
                            nc.sync.dma_start(
                                out=wk_r[:],
                                in_=wk_p[:, et * P:(et + 1) * P].rearrange(
                                    "(dt p) e -> p dt e", p=P))
                        ps = psb[ch * 2 + (et % 2)]
                        for d in range(DT):
                            nc.tensor.matmul(
                                ps[:],
                                wk_r[:, d, :],
                                xT_keys[:, d, 0, ch * 512:(ch + 1) * 512],
                                start=(d == 0), stop=(d == DT - 1))
                        nc.vector.tensor_copy(
                            kT[:, et, ch * 512:(ch + 1) * 512], ps[:])

                # A3: v[i, e] = sum_d x_key[i, d] * Wv[d, e]   (8 psum banks)
                for eb in range(2):
                    for d in range(DT):
                        wv_r = stage_pool.tile([P, 512], F32R, tag="wvr",
                                               name=f"wvr{eb}_{d}", bufs=3)
                        nc.sync.dma_start(
                            out=wv_r[:],
                            in_=wv_p[d * P:(d + 1) * P,
                                     eb * 512:(eb + 1) * 512])
                        for st in range(HKT):
                            nc.tensor.matmul(
                                psb[st][:],
                                xT_keys[:, d, 0, st * P:(st + 1) * P],
                                wv_r[:],
                                start=(d == 0), stop=(d == DT - 1))
                    for st in range(HKT):
                        nc.vector.tensor_copy(
                            vv[:, st, eb * 512:(eb + 1) * 512], psb[st][:])

                # A4: qT[e, s] = sum_d Wq[d, e] * x[s, d]  (all queries)
                for et in range(ET):
                    wq_r = stage_pool.tile([P, DT, P], F32R, tag="wr",
                                           name=f"wqr{et}", bufs=3)
                    nc.sync.dma_start(
                        out=wq_r[:],
                        in_=wq_p[:, et * P:(et + 1) * P].rearrange(
                            "(dt p) e -> p dt e", p=P))
                    pss = [psb[ch * 2 + (et % 2)] for ch in range(4)]
                    for d in range(DT):
                        for ch in range(4):
                            nc.tensor.matmul(
                                pss[ch][:],
                                wq_r[:, d, :],
                                xT[:, d, ch * 512:(ch + 1) * 512],
                                start=(d == 0), stop=(d == DT - 1))
                    for ch in range(4):
                        nc.vector.tensor_copy(
                            qT[:, et, ch * 512:(ch + 1) * 512], pss[ch][:])

            # ================= Phase B: causal attention =================
            with (
                tc.tile_pool(name="at_pool", bufs=2) as at_pool,
                tc.tile_pool(name="atT_pool", bufs=8) as atT_pool,
                tc.tile_pool(name="ob_pool", bufs=2) as ob_pool,
                tc.tile_pool(name="psS_pool", bufs=2, space="PSUM") as psS_pool,
                tc.tile_pool(name="psA_pool", bufs=2, space="PSUM") as psA_pool,
                tc.tile_pool(name="psO_pool", bufs=1, space="PSUM") as psO_pool,
            ):
                for j in range(NQB):
                    nkb = j // 2 + 1          # valid compacted key blocks
                    ncols = nkb * P
                    csizes = _chunks(ncols)
                    attn = at_pool.tile([P, HKT * P], BF16, tag="attn",
                                        name=f"attn{j}")

                    # scores = qT[:, j-block]^T @ kT  (contract over e);
                    # mask is added into PSUM, exp reads PSUM directly.
                    lo = 0
                    for ci, csz in enumerate(csizes):
                        psS = psS_pool.tile([P, 512], F32, tag=f"psS{ci % 2}",
                                            name=f"psS{j}_{ci}")
                        for et in range(ET):
                            nc.tensor.matmul(
                                psS[:, :csz],
                                qT[:, et, j * P:(j + 1) * P],
                                kT[:, et, lo:lo + csz],
                                start=(et == 0), stop=(et == ET - 1))
                        if ci == len(csizes) - 1:
                            nc.vector.tensor_add(
                                psS[:, csz - P:csz],
                                psS[:, csz - P:csz],
                                mask_sb[:, j % 2, :])
                        nc.scalar.activation(
                            attn[:, lo:lo + csz], psS[:, :csz],
                            mybir.ActivationFunctionType.Exp,
                            scale=INV_SQRT_D,
                            accum_out=l_parts[:, j, ci:ci + 1])
                        lo += csz

                    # o = attn @ v   (transpose attn blocks, contract keys)
                    atTs = []
                    for kb in range(nkb):
                        psA = psA_pool.tile([P, P], BF16, tag="psA",
                                            name=f"psA{j}_{kb}")
                        atT = atT_pool.tile([P, P], BF16, tag="atT",
                                            name=f"atT{j}_{kb}")
                        nc.tensor.transpose(
                            psA[:], attn[:, kb * P:(kb + 1) * P], ident_bf[:])
                        nc.vector.tensor_copy(atT[:], psA[:])
                        atTs.append(atT)
                    psO = [psO_pool.tile([P, 512], F32, tag=f"psO{eb}",
                                         name=f"psO{j}_{eb}")
                           for eb in range(2)]
                    for kb in range(nkb):
                        for eb in range(2):
                            nc.tensor.matmul(
                                psO[eb][:],
                                atTs[kb][:],
                                vv[:, kb, eb * 512:(eb + 1) * 512],
                                start=(kb == 0), stop=(kb == nkb - 1))
                    for eb in range(2):
                        o_sb = ob_pool.tile([P, 512], F32, tag=f"o{eb}",
                                            name=f"o{j}_{eb}")
                        nc.vector.tensor_copy(o_sb[:], psO[eb][:])
                        nc.sync.dma_start(
                            out=o_p[j * P:(j + 1) * P,
                                    eb * 512:(eb + 1) * 512],
                            in_=o_sb[:])
                nc.vector.tensor_add(l_sum[:], l_parts[:, :, 0],
                                     l_parts[:, :, 1])
                nc.sync.dma_start(out=l_p[:], in_=l_sum[:])
    nc.finalize()
    return nc


def _boundary_masks(c):
    """mask[row, par, i]: 0 if compacted key i is causally valid for local
    query row `row` of an even (par=0) / odd (par=1) query block, else -1e30.

    For parity-1 cores, x rows arrive pair-swapped, so the query at local
    position `row` is global row 128*j + r_local with
    r_local = row+1 (even row) / row-1 (odd row). Key i is global row
    256*(j//2) + 2*i + c. Valid iff 2*i + c <= par*128 + r_local.
    """
    mask = np.full((P, 2, P), NEG, dtype=np.float32)
    for row in range(P):
        r_local = row if c == 0 else (row + 1 if row % 2 == 0 else row - 1)
        for par in range(2):
            lim = (par * P + r_local - c) // 2
            if lim >= 0:
                mask[row, par, :min(lim + 1, P)] = 0.0
    return mask


_PAIRSWAP = np.arange(S).reshape(-1, 2)[:, ::-1].reshape(-1)


def _make_in_maps(x, Wq, Wk, Wv):
    x = np.asarray(x, dtype=np.float32)
    Wq = np.ascontiguousarray(np.asarray(Wq, dtype=np.float32))
    Wk = np.ascontiguousarray(np.asarray(Wk, dtype=np.float32))
    Wv = np.ascontiguousarray(np.asarray(Wv, dtype=np.float32))
    masks = [_boundary_masks(0), _boundary_masks(1)]
    in_maps = []
    for core in range(8):
        b, c = core // 2, core % 2
        xb = x[b] if c == 0 else x[b][_PAIRSWAP]
        xt = xb.T.reshape(DT, P, 2, S // 2).transpose(0, 2, 1, 3)
        in_maps.append({
            "xt": np.ascontiguousarray(xt.reshape(DT * 2 * P, S // 2)),
            "wq": Wq, "wk": Wk, "wv": Wv,
            "mask": masks[c],
        })
    return in_maps


def _combine(res):
    out = np.empty((B, S, D), dtype=np.float32)
    for b in range(B):
        r0, r1 = res.results[2 * b], res.results[2 * b + 1]

        def stat(r, key):
            return np.ascontiguousarray(r[key].T).reshape(S, 1)
        o0 = r0["o"].astype(np.float64)
        l0 = stat(r0, "l").astype(np.float64)
        # parity-1 core computed on pair-swapped query rows; swap back
        o1 = r1["o"][_PAIRSWAP].astype(np.float64)
        l1 = stat(r1, "l")[_PAIRSWAP].astype(np.float64)
        out[b] = ((o0 + o1) / (l0 + l1)).astype(np.float32)
    return out


def kernel(x, Wq, Wk, Wv):
    global _CACHED_NC
    if _CACHED_NC is None:
        _CACHED_NC = build_nc()
    in_maps = _make_in_maps(x, Wq, Wk, Wv)
    res = run_bass_kernel_spmd(_CACHED_NC, in_maps, list(range(8)))
    return _combine(res)


# revision 24
# speedup vs baseline: 1.3042x; 1.3042x over previous
"""Causal self-attention (B=4, S=2048, D=1024, single head, fp32) on 8 trn2
NeuronCores.

Sharding: core 2*b + c handles batch b with the parity-c half of the keys
(global key rows 2*i + c), over ALL queries — a flash-attention split over
the key dimension. Each core returns unnormalized softmax numerators
o = sum_k exp(s*scale) v plus per-row l = sum exp; the host combines the
two key-halves exactly (no max subtraction needed: |logits| <~ 6).

SPMD trick: one program serves both parities. The host pair-swaps the rows
of x for odd cores (rows [1,0,3,2,...]), so each core's keys sit at even
row positions and the on-chip stride-2 access pattern is parity-free. The
causal boundary masks (which depend on the parity) ship as a small
per-core input; the host pair-swaps the outputs of odd cores back.

x ships pre-transposed from the host (layout prep is host-side, like the
pair-swap), so the kernel spends no PE/DVE cycles transposing; everything
runs in float32r (full PE rate at free-dim >= 256) directly off DMA —
f32r is bit-identical to f32, so no cast pass is needed. Softmax exp runs
on the scalar engine straight out of PSUM with the causal mask pre-added
into PSUM; attn @ v runs in bf16.
"""
import math
import numpy as np

import concourse.bacc as bacc
import concourse.mybir as mybir
from concourse import tile
from concourse.masks import make_identity
from concourse.bass_utils import run_bass_kernel_spmd

B, S, D = 4, 2048, 1024
P = 128
DT = D // P          # 8 d-tiles (contraction)
ET = D // P          # 8 e-tiles (output feature)
ST = S // P          # 16 s-tiles (sequence)
HKT = ST // 2        # 8 compacted key tiles per core
NQB = S // P         # 16 query blocks
INV_SQRT_D = 1.0 / math.sqrt(D)
NEG = -1e30

F32 = mybir.dt.float32
F32R = mybir.dt.float32r
BF16 = mybir.dt.bfloat16

_CACHED_NC = None


def _chunks(ncols):
    """Split ncols (multiple of 128) into <=512 chunks, all >=256 when
    possible (f32r matmul runs 4 cyc/row below 256 moving rows)."""
    if ncols <= 512:
        return [ncols]
    if ncols <= 896:
        return [ncols - 384, 384]
    return [512, ncols - 512]


def build_nc():
    nc = bacc.Bacc("TRN2", target_bir_lowering=False)
    # x^T shipped chunk-major: block (dt*2+h) holds x^T[dt*128:(dt+1)*128,
    # h*1024:(h+1)*1024] as a contiguous 512KB run for full-rate DMA.
    xt_p = nc.declare_dram_parameter("xt", [DT * 2 * P, S // 2], F32R,
                                     isOutput=False)
    wq_p = nc.declare_dram_parameter("wq", [D, D], F32R, isOutput=False)
    wk_p = nc.declare_dram_parameter("wk", [D, D], F32R, isOutput=False)
    wv_p = nc.declare_dram_parameter("wv", [D, D], F32R, isOutput=False)
    mask_p = nc.declare_dram_parameter("mask", [P, 2, P], F32, isOutput=False)
    o_p = nc.declare_dram_parameter("o", [S, D], F32, isOutput=True)
    l_p = nc.declare_dram_parameter("l", [P, NQB], F32, isOutput=True)
    dbg_p = nc.declare_dram_parameter("dbg", [P, 1], F32, isOutput=True)

    with tile.TileContext(nc) as tc:
        # ---- persistent pools (bottom of SBUF stack) ----
        with (
            tc.tile_pool(name="qT_pool", bufs=1) as qT_pool,
            tc.tile_pool(name="kT_pool", bufs=1) as kT_pool,
            tc.tile_pool(name="v_pool", bufs=1) as v_pool,
            tc.tile_pool(name="const_pool", bufs=1) as const_pool,
        ):
            qT = qT_pool.tile([P, ET, S], F32R)        # [e_p, et, s_q] 64KB/p
            kT = kT_pool.tile([P, ET, HKT * P], F32R)  # [e_p, et, s_k] 32KB/p
            vv = v_pool.tile([P, HKT, D], BF16)        # [s_k_p, st, e] 16KB/p
            ident_bf = const_pool.tile([P, P], BF16)
            mask_sb = const_pool.tile([P, 2, P], F32)
            l_parts = const_pool.tile([P, NQB, 2], F32)
            l_sum = const_pool.tile([P, NQB], F32)
            warm = const_pool.tile([P, 512], BF16)
            dbg_sb = const_pool.tile([P, 1], F32)
            nc.vector.memset(warm[:], 0.25)
            make_identity(nc, ident_bf[:])
            nc.gpsimd.memset(l_parts[:], 0.0)
            nc.sync.dma_start(out=mask_sb[:], in_=mask_p[:])

            # ================= Phase A: x^T + projections =================
            with (
                tc.tile_pool(name="xT_pool", bufs=1) as xT_pool,
                tc.tile_pool(name="stage_pool", bufs=2) as stage_pool,
                tc.tile_pool(name="psA_all", bufs=1, space="PSUM") as psAll,
            ):
                xT = xT_pool.tile([P, DT, S], F32R)    # [d_p, dt, s] 64KB/p
                psb = [psAll.tile([P, 512], F32, tag=f"b{i}", name=f"psb{i}")
                       for i in range(8)]

                # A1: load pre-transposed x. First half-columns go on the
                # sync queue (interleaved with the first wk slices) so A2's
                # ch=0 pass can start as soon as possible; the second half
                # loads in the background on the gpsimd DMA queue.
                wk_t = {}
                for ch in range(2):
                    for et in range(ET):
                        wk_t[(ch, et)] = stage_pool.tile(
                            [P, DT, P], F32R, tag="wr",
                            name=f"wkr{ch}_{et}", bufs=3)
                for et in range(3):
                    nc.sync.dma_start(
                        out=wk_t[(0, et)][:],
                        in_=wk_p[:, et * P:(et + 1) * P].rearrange(
                            "(dt p) e -> p dt e", p=P))
                for h in range(2):
                    for dt in range(DT):
                        nc.gpsimd.dma_start(
                            out=xT[:, dt, h * 1024:(h + 1) * 1024],
                            in_=xt_p[(dt * 2 + h) * P:(dt * 2 + h + 1) * P,
                                     :])

                # PE warmup: keeps the tensor engine busy (and its clock
                # ramping) while the first x/wk chunks stream in. The psum
                # result feeds a dummy output so it isn't dead-code.
                for w in range(24):
                    nc.tensor.matmul(psb[7][:], warm[:, :P], warm[:],
                                     start=(w == 0), stop=(w == 23))
                nc.vector.tensor_copy(dbg_sb[:], psb[7][:, :1])
                nc.sync.dma_start(out=dbg_p[:], in_=dbg_sb[:])

                # even-position (this core's keys) stride-2 view of xT
                xT_keys = xT.rearrange("p d (s two) -> p d two s", two=2)

                # A2: kT[e, i] = sum_d Wk[d, e] * x_key[i, d].  ch-outer so
                # the first pass only needs the first half of x.
                for ch in range(2):
                    for et in range(ET):
                        wk_r = wk_t[(ch, et)]
                        if not (ch == 0 and et < 3):
                            nc.sync.dma_start(
                                out=wk_r[:],
                                in_=wk_p[:, et * P:(et + 1) * P].rearrange(
                                    "(dt p) e -> p dt e", p=P))
                        ps = psb[ch * 2 + (et % 2)]
                        for d in range(DT):
                            nc.tensor.matmul(
                                ps[:],
                                wk_r[:, d, :],
                                xT_keys[:, d, 0, ch * 512:(ch + 1) * 512],
                                start=(d == 0), stop=(d == DT - 1))
                        nc.vector.tensor_copy(
                            kT[:, et, ch * 512:(ch + 1) * 512], ps[:])

                # A3: v[i, e] = sum_d x_key[i, d] * Wv[d, e]   (8 psum banks)
                for eb in range(2):
                    for d in range(DT):
                        wv_r = stage_pool.tile([P, 512], F32R, tag="wvr",
                                               name=f"wvr{eb}_{d}", bufs=3)
                        nc.gpsimd.dma_start(
                            out=wv_r[:],
                            in_=wv_p[d * P:(d + 1) * P,
                                     eb * 512:(eb + 1) * 512])
                        for st in range(HKT):
                            nc.tensor.matmul(
                                psb[st][:],
                                xT_keys[:, d, 0, st * P:(st + 1) * P],
                                wv_r[:],
                                start=(d == 0), stop=(d == DT - 1))
                    for st in range(HKT):
                        nc.vector.tensor_copy(
                            vv[:, st, eb * 512:(eb + 1) * 512], psb[st][:])

                # A4: qT[e, s] = sum_d Wq[d, e] * x[s, d]  (all queries)
                for et in range(ET):
                    wq_r = stage_pool.tile([P, DT, P], F32R, tag="wr",
                                           name=f"wqr{et}", bufs=3)
                    nc.sync.dma_start(
                        out=wq_r[:],
                        in_=wq_p[:, et * P:(et + 1) * P].rearrange(
                            "(dt p) e -> p dt e", p=P))
                    pss = [psb[ch * 2 + (et % 2)] for ch in range(4)]
                    for d in range(DT):
                        for ch in range(4):
                            nc.tensor.matmul(
                                pss[ch][:],
                                wq_r[:, d, :],
                                xT[:, d, ch * 512:(ch + 1) * 512],
                                start=(d == 0), stop=(d == DT - 1))
                    for ch in range(4):
                        nc.vector.tensor_copy(
                            qT[:, et, ch * 512:(ch + 1) * 512], pss[ch][:])

            # ================= Phase B: causal attention =================
            with (
                tc.tile_pool(name="at_pool", bufs=2) as at_pool,
                tc.tile_pool(name="atT_pool", bufs=8) as atT_pool,
                tc.tile_pool(name="ob_pool", bufs=2) as ob_pool,
                tc.tile_pool(name="psS_pool", bufs=2, space="PSUM") as psS_pool,
                tc.tile_pool(name="psA_pool", bufs=2, space="PSUM") as psA_pool,
                tc.tile_pool(name="psO_pool", bufs=1, space="PSUM") as psO_pool,
            ):
                for j in range(NQB):
                    nkb = j // 2 + 1          # valid compacted key blocks
                    ncols = nkb * P
                    csizes = _chunks(ncols)
                    attn = at_pool.tile([P, HKT * P], BF16, tag="attn",
                                        name=f"attn{j}")

                    # scores = qT[:, j-block]^T @ kT  (contract over e);
                    # mask is added into PSUM, exp reads PSUM directly.
                    lo = 0
                    for ci, csz in enumerate(csizes):
                        psS = psS_pool.tile([P, 512], F32, tag=f"psS{ci % 2}",
                                            name=f"psS{j}_{ci}")
                        for et in range(ET):
                            nc.tensor.matmul(
                                psS[:, :csz],
                                qT[:, et, j * P:(j + 1) * P],
                                kT[:, et, lo:lo + csz],
                                start=(et == 0), stop=(et == ET - 1))
                        if ci == len(csizes) - 1:
                            nc.vector.tensor_add(
                                psS[:, csz - P:csz],
                                psS[:, csz - P:csz],
                                mask_sb[:, j % 2, :])
                        nc.scalar.activation(
                            attn[:, lo:lo + csz], psS[:, :csz],
                            mybir.ActivationFunctionType.Exp,
                            scale=INV_SQRT_D,
                            accum_out=l_parts[:, j, ci:ci + 1])
                        lo += csz

                    # o = attn @ v   (transpose attn blocks, contract keys)
                    atTs = []
                    for kb in range(nkb):
                        psA = psA_pool.tile([P, P], BF16, tag="psA",
                                            name=f"psA{j}_{kb}")
                        atT = atT_pool.tile([P, P], BF16, tag="atT",
                                            name=f"atT{j}_{kb}")
                        nc.tensor.transpose(
                            psA[:], attn[:, kb * P:(kb + 1) * P], ident_bf[:])
                        nc.vector.tensor_copy(atT[:], psA[:])
                        atTs.append(atT)
                    psO = [psO_pool.tile([P, 512], F32, tag=f"psO{eb}",
                                         name=f"psO{j}_{eb}")
                           for eb in range(2)]
                    for kb in range(nkb):
                        for eb in range(2):
                            nc.tensor.matmul(
                                psO[eb][:],
                                atTs[kb][:],
                                vv[:, kb, eb * 512:(eb + 1) * 512],
                                start=(kb == 0), stop=(kb == nkb - 1))
                    for eb in range(2):
                        o_sb = ob_pool.tile([P, 512], F32, tag=f"o{eb}",
                                            name=f"o{j}_{eb}")
                        nc.vector.tensor_copy(o_sb[:], psO[eb][:])
                        nc.sync.dma_start(
                            out=o_p[j * P:(j + 1) * P,
                                    eb * 512:(eb + 1) * 512],
                            in_=o_sb[:])
                nc.vector.tensor_add(l_sum[:], l_parts[:, :, 0],
                                     l_parts[:, :, 1])
                nc.sync.dma_start(out=l_p[:], in_=l_sum[:])
    nc.finalize()
    return nc


def _boundary_masks(c):
    """mask[row, par, i]: 0 if compacted key i is causally valid for local
    query row `row` of an even (par=0) / odd (par=1) query block, else -1e30.

    For parity-1 cores, x rows arrive pair-swapped, so the query at local
    position `row` is global row 128*j + r_local with
    r_local = row+1 (even row) / row-1 (odd row). Key i is global row
    256*(j//2) + 2*i + c. Valid iff 2*i + c <= par*128 + r_local.
    """
    mask = np.full((P, 2, P), NEG, dtype=np.float32)
    for row in range(P):
        r_local = row if c == 0 else (row + 1 if row % 2 == 0 else row - 1)
        for par in range(2):
            lim = (par * P + r_local - c) // 2
            if lim >= 0:
                mask[row, par, :min(lim + 1, P)] = 0.0
    return mask


_PAIRSWAP = np.arange(S).reshape(-1, 2)[:, ::-1].reshape(-1)


def _make_in_maps(x, Wq, Wk, Wv):
    x = np.asarray(x, dtype=np.float32)
    Wq = np.ascontiguousarray(np.asarray(Wq, dtype=np.float32))
    Wk = np.ascontiguousarray(np.asarray(Wk, dtype=np.float32))
    Wv = np.ascontiguousarray(np.asarray(Wv, dtype=np.float32))
    masks = [_boundary_masks(0), _boundary_masks(1)]
    in_maps = []
    for core in range(8):
        b, c = core // 2, core % 2
        xb = x[b] if c == 0 else x[b][_PAIRSWAP]
        xt = xb.T.reshape(DT, P, 2, S // 2).transpose(0, 2, 1, 3)
        in_maps.append({
            "xt": np.ascontiguousarray(xt.reshape(DT * 2 * P, S // 2)),
            "wq": Wq, "wk": Wk, "wv": Wv,
            "mask": masks[c],
        })
    return in_maps


def _combine(res):
    out = np.empty((B, S, D), dtype=np.float32)
    for b in range(B):
        r0, r1 = res.results[2 * b], res.results[2 * b + 1]

        def stat(r, key):
            return np.ascontiguousarray(r[key].T).reshape(S, 1)
        o0 = r0["o"].astype(np.float64)
        l0 = stat(r0, "l").astype(np.float64)
        # parity-1 core computed on pair-swapped query rows; swap back
        o1 = r1["o"][_PAIRSWAP].astype(np.float64)
        l1 = stat(r1, "l")[_PAIRSWAP].astype(np.float64)
        out[b] = ((o0 + o1) / (l0 + l1)).astype(np.float32)
    return out


def kernel(x, Wq, Wk, Wv):
    global _CACHED_NC
    if _CACHED_NC is None:
        _CACHED_NC = build_nc()
    in_maps = _make_in_maps(x, Wq, Wk, Wv)
    res = run_bass_kernel_spmd(_CACHED_NC, in_maps, list(range(8)))
    return _combine(res)
